# revision 13
# baseline (speedup 1.0000x reference)
"""Trainium2 Bass kernel for EquivariantGraphConv message passing.

Math (reference):
    scalar = x[:,0,:]; vector = x[:,1:,:].reshape(N, 3H)
    scalar_out = scalar @ Wsr.T + b + segsum(scalar[col] @ Wsrel.T, row)
    vector_out = vector @ Wvr.T + segsum(vector[col] @ Wvrel.T, row)

Key identity used: the edge transform is linear, so
    segsum(feat[col] @ W.T, row) == segsum(feat[col], row) @ W.T
We therefore aggregate the raw 512-dim node features per destination first
(16x fewer matmul FLOPs), then apply all four weight matrices per *node*.

Sharding: destinations are sharded across the 8 cores (1280 nodes each, in
10 chunks of 128).  Edges are sorted by destination on the host, so each
core only consumes its own edge shard and no cross-core reduction is
needed.  Each core gathers source features from a replicated padded node
table in DRAM with one big indirect DMA per chunk, builds one-hot
"selection" matrices on the vector engine (row_in_chunk == iota) and
matmul-accumulates P^T @ G into PSUM to realize the segment sum.

v4:
  - per-chunk edge-tile counts T_vec[j] (chunks sorted by edge count per
    core on the host; T_vec = per-position max across cores) instead of a
    uniform worst-case T: ~7% less gather traffic and PE work
  - one batched is_equal per chunk (stride-0 broadcast dims) instead of
    T small ones: DVE one-hot cost is fixed-overhead dominated
  - index tensors load first on the scalar HWDGE queue; chunk 0's slice
    is its own tiny DMA
  - gather indices sorted by source id within each chunk (HBM locality)
  - PSUM->SBUF copies on the activation engine; output written in bf16

v5:
  - chunks processed smallest-first (earlier first matmul)
  - root transforms (x @ Wroot.T + bias), which don't depend on edges,
    are computed for all chunks up front while the first gathers are
    still in flight -- the PE is otherwise idle there; stage 2 then only
    runs the rel matmuls and one DVE add of the precomputed root
"""

import os
import sys

sys.path.insert(0, "/opt/trn_rl_repo")

import numpy as np
import ml_dtypes

import concourse.bass as bass
import concourse.mybir as mybir
import concourse.tile as tile
from concourse.bacc import Bacc
from concourse.bass_utils import run_bass_kernel_spmd

N_NODES = 10000
N_EDGES = 160000
H = 128
F = 4 * H            # 512 features per node (scalar 128 + vector 384)
P = 128              # partitions
NP_PAD = 10240       # padded node count (80 chunks of 128)
N_CORES = 8
NODES_PER_CORE = NP_PAD // N_CORES       # 1280
CHUNKS_PER_CORE = NODES_PER_CORE // P    # 10
N_CHUNKS = NP_PAD // P                   # 80
ZERO_ROW = N_NODES                       # padded zero row used by dummy edges
GQ = 1024                                # max gather descriptors per instr
WQ = GQ // 16

CFG = "v4-varT"

# test.py hooks
PROFILE = {"on": False, "trace_cores": None, "last": None}

_prog_cache = {}

BF16 = mybir.dt.bfloat16
NP_BF16 = ml_dtypes.bfloat16


def _nq(t):
    return (t * P + GQ - 1) // GQ


def _build_program(T_vec):
    """Build the (SPMD, per-core-identical) Bass program."""
    nc = Bacc("TRN2", num_swdge_queues=4)
    f32 = mybir.dt.float32

    T_max = max(T_vec)
    NQ_vec = [_nq(t) for t in T_vec]
    CO = np.concatenate([[0], np.cumsum([nq * WQ for nq in NQ_vec])])  # cols offs
    RO = np.concatenate([[0], np.cumsum(T_vec)])                       # rr offs

    xg = nc.dram_tensor("xg", [NP_PAD, F], BF16, kind="ExternalInput")
    cols0 = nc.dram_tensor("cols0", [P, int(CO[1])], mybir.dt.int16,
                           kind="ExternalInput")
    colsR = nc.dram_tensor("colsR", [P, int(CO[-1] - CO[1])],
                           mybir.dt.int16, kind="ExternalInput")
    rr = nc.dram_tensor("rr", [P, int(RO[-1])], BF16, kind="ExternalInput")
    xt = nc.dram_tensor("xt", [P, 4 * NODES_PER_CORE], BF16,
                        kind="ExternalInput")
    wsrel = nc.dram_tensor("wsrel", [P, H], BF16, kind="ExternalInput")
    wsroot = nc.dram_tensor("wsroot", [P, H], BF16, kind="ExternalInput")
    wvrel = nc.dram_tensor("wvrel", [P, 3 * 384], BF16, kind="ExternalInput")
    wvroot = nc.dram_tensor("wvroot", [P, 3 * 384], BF16, kind="ExternalInput")
    bias = nc.dram_tensor("bias", [P, H], f32, kind="ExternalInput")
    iota = nc.dram_tensor("iota", [P, P], BF16, kind="ExternalInput")
    ident = nc.dram_tensor("ident", [P, P], BF16, kind="ExternalInput")
    out = nc.dram_tensor("out", [NODES_PER_CORE, F], BF16, kind="ExternalOutput")

    with tile.TileContext(nc) as tc:
        with (
            tc.tile_pool(name="consts", bufs=1) as cpool,
            tc.tile_pool(name="edges", bufs=6) as epool,
            tc.tile_pool(name="gbuf", bufs=5) as gpool,
            tc.tile_pool(name="work", bufs=4) as wpool,
            tc.tile_pool(name="pagg", bufs=2, space="PSUM") as pagg,
            tc.tile_pool(name="pmisc", bufs=2, space="PSUM") as pmisc,
            tc.tile_pool(name="roots", bufs=CHUNKS_PER_CORE) as rpool,
            tc.tile_pool(name="proot", bufs=2, space="PSUM") as prpool,
        ):
            # prewarm the SWDGE gather pipeline (lib load + queue init)
            # with a dummy 128-descriptor gather of the zero row
            dummy_idx = cpool.tile([P, 8], mybir.dt.int16)
            nc.gpsimd.memset(dummy_idx[:], 0)
            dummy_g = cpool.tile([P, F], BF16)
            nc.gpsimd.dma_gather(
                dummy_g[:].rearrange("p (t f) -> p t f", f=F),
                xg[:], dummy_idx[:], P, P, F, queue_num=0)

            # index tensors first on the scalar HWDGE queue (chunk 0's cols
            # slice alone, tiny), then what the root-warmup needs (root
            # weights, bias, xt), then the rest
            cols0_sb = cpool.tile([P, int(CO[1])], mybir.dt.int16)
            nc.scalar.dma_start(cols0_sb[:], cols0[:])
            rr_all = cpool.tile([P, int(RO[-1])], BF16)
            nc.scalar.dma_start(rr_all[:], rr[:])
            wsroot_sb = cpool.tile([P, H], BF16)
            nc.scalar.dma_start(wsroot_sb[:], wsroot[:])
            wvroot_sb = cpool.tile([P, 3 * 384], BF16)
            nc.scalar.dma_start(wvroot_sb[:], wvroot[:])
            bias_sb = cpool.tile([P, H], f32)
            nc.scalar.dma_start(bias_sb[:], bias[:])
            xt_sb = cpool.tile([P, 4 * NODES_PER_CORE], BF16)
            nc.scalar.dma_start(xt_sb[:], xt[:])
            colsR_sb = cpool.tile([P, int(CO[-1] - CO[1])], mybir.dt.int16)
            nc.scalar.dma_start(colsR_sb[:], colsR[:])
            iota_sb = cpool.tile([P, P], BF16)
            nc.scalar.dma_start(iota_sb[:], iota[:])
            ident_sb = cpool.tile([P, P], BF16)
            nc.scalar.dma_start(ident_sb[:], ident[:])
            wsrel_sb = cpool.tile([P, H], BF16)
            nc.scalar.dma_start(wsrel_sb[:], wsrel[:])
            wvrel_sb = cpool.tile([P, 3 * 384], BF16)
            nc.scalar.dma_start(wvrel_sb[:], wvrel[:])

            LAG = 2  # stage-2 for chunk c-LAG runs amid stage-1 of chunk c
            agg_tiles = {}
            root_tiles = {}
            qctr = [0]

            # root transforms for every chunk while the gathers warm up:
            # root = x @ Wroot.T (+ bias on the scalar half)
            for c in range(CHUNKS_PER_CORE):
                root_ps = prpool.tile([P, F], f32, tag="rootps")
                nc.tensor.matmul(out=root_ps[:, 0:H],
                                 lhsT=xt_sb[:, c * P:(c + 1) * P],
                                 rhs=wsroot_sb[:],
                                 start=True, stop=True)
                for kc in range(3):
                    nc.tensor.matmul(
                        out=root_ps[:, H:F],
                        lhsT=xt_sb[:, (1 + kc) * NODES_PER_CORE + c * P:
                                      (1 + kc) * NODES_PER_CORE + (c + 1) * P],
                        rhs=wvroot_sb[:, kc * 384:(kc + 1) * 384],
                        start=(kc == 0), stop=(kc == 2))
                root_sb = rpool.tile([P, F], BF16, tag="root")
                nc.vector.tensor_add(root_sb[:, 0:H], root_ps[:, 0:H],
                                     bias_sb[:])
                nc.scalar.copy(root_sb[:, H:F], root_ps[:, H:F])
                root_tiles[c] = root_sb

            def stage1(c):
                T = T_vec[c]
                NQ = NQ_vec[c]
                if c == 0:
                    cols_sb = cols0_sb[:, :]
                else:
                    cols_sb = colsR_sb[:, int(CO[c] - CO[1]):int(CO[c + 1] - CO[1])]
                rr_sb = rr_all[:, int(RO[c]):int(RO[c + 1])]

                # gather: edge i -> G[i % 128, i // 128, :] = xg[cols_flat[i], :]
                G = gpool.tile([P, T_max * F], BF16, tag="G")
                for q in range(NQ):
                    nidx = min(GQ, T * P - q * GQ)
                    nslots = nidx // P
                    nc.gpsimd.dma_gather(
                        G[:, q * (GQ // P) * F:
                             (q * (GQ // P) + nslots) * F]
                        .rearrange("p (t f) -> p t f", f=F),
                        xg[:],
                        cols_sb[:, q * WQ:(q + 1) * WQ],
                        nidx,
                        nidx,
                        F,
                        queue_num=qctr[0] % 4,
                    )
                    qctr[0] += 1

                # one-hot P[p, t*128 + d] = (rr[p, t] == d), one batched
                # is_equal over all T tiles (broadcast via stride-0 dims)
                Pm = epool.tile([P, T_max * P], BF16, tag="P")
                nc.vector.tensor_tensor(
                    out=Pm[:, :T * P].rearrange("p (t d) -> p t d", d=P),
                    in0=rr_sb.rearrange("p (t o) -> p t o", o=1)
                        .to_broadcast([P, T, P]),
                    in1=iota_sb[:].rearrange("p (o d) -> p o d", o=1)
                        .to_broadcast([P, T, P]),
                    op=mybir.AluOpType.is_equal,
                )

                # segment-sum: agg[d, f] = sum_t P_t^T @ G_t
                agg_ps = pagg.tile([P, F], f32, tag="agg")
                for t in range(T):
                    nc.tensor.matmul(
                        out=agg_ps[:],
                        lhsT=Pm[:, t * P:(t + 1) * P],
                        rhs=G[:, t * F:(t + 1) * F],
                        start=(t == 0),
                        stop=(t == T - 1),
                    )
                agg_sb = wpool.tile([P, F], BF16, tag="aggsb")
                nc.scalar.copy(agg_sb[:], agg_ps[:])
                agg_tiles[c] = agg_sb

            def stage2(c):
                agg_sb = agg_tiles.pop(c)
                # transpose agg -> aggT[f, d] (4 PE transposes of 128x128)
                aggT_ps = pmisc.tile([P, F], BF16, tag="aggT")
                for fc in range(4):
                    nc.tensor.transpose(
                        out=aggT_ps[:, fc * P:(fc + 1) * P],
                        in_=agg_sb[:, fc * P:(fc + 1) * P],
                        identity=ident_sb[:],
                    )
                aggT_sb = wpool.tile([P, F], BF16, tag="aggTsb")
                nc.scalar.copy(aggT_sb[:], aggT_ps[:])

                # stage 2 (rel parts only; root was precomputed):
                #   out[d, :128] = agg_s @ WsrelT + root_s
                #   out[d, 128:] = agg_v @ WvrelT + root_v
                osv_ps = pmisc.tile([P, F], f32, tag="osv")
                nc.tensor.matmul(out=osv_ps[:, 0:H],
                                 lhsT=aggT_sb[:, 0:P], rhs=wsrel_sb[:],
                                 start=True, stop=True)
                for kc in range(3):
                    nc.tensor.matmul(
                        out=osv_ps[:, H:F],
                        lhsT=aggT_sb[:, (1 + kc) * P:(2 + kc) * P],
                        rhs=wvrel_sb[:, kc * 384:(kc + 1) * 384],
                        start=(kc == 0), stop=(kc == 2))

                out_sb = wpool.tile([P, F], BF16, tag="outsb")
                nc.vector.tensor_tensor(out=out_sb[:], in0=osv_ps[:],
                                        in1=root_tiles.pop(c)[:],
                                        op=mybir.AluOpType.add)
                nc.sync.dma_start(out[c * P:(c + 1) * P, :], out_sb[:])

            for c in range(CHUNKS_PER_CORE + LAG):
                if c < CHUNKS_PER_CORE:
                    stage1(c)
                if c >= LAG:
                    stage2(c - LAG)

    nc.finalize()
    return nc


def _get_program(T_vec):
    key = tuple(T_vec)
    if key not in _prog_cache:
        _prog_cache[key] = _build_program(list(T_vec))
    return _prog_cache[key]


def kernel(x, edge_index, W_scalar_rel, W_scalar_root, b_scalar_root,
           W_vector_rel, W_vector_root):
    x = np.asarray(x, dtype=np.float32)
    n = x.shape[0]
    assert n == N_NODES, x.shape
    row = np.asarray(edge_index[0], dtype=np.int64)
    col = np.asarray(edge_index[1], dtype=np.int64)

    # ---- host-side shard construction ----
    # sort edges by (dest chunk, source id): chunk-grouped for the local
    # segment sum, source-ascending within a chunk so the gather's HBM
    # reads walk mostly-increasing addresses
    order = np.lexsort((col, row // P))
    row_s = row[order]
    col_s = col[order]
    chunk_of = row_s // P
    bounds = np.searchsorted(chunk_of, np.arange(N_CHUNKS + 1))
    counts = np.diff(bounds)                       # edges per original chunk
    T_chunk = np.maximum(1, np.ceil(counts / P).astype(int))

    # per-core chunk processing order: smallest chunks first (earliest
    # first matmul); T_vec[j] is the max of the j-th smallest T across
    # cores (shared SPMD program)
    T_mat = T_chunk.reshape(N_CORES, CHUNKS_PER_CORE)
    perm = np.argsort(T_mat, axis=1, kind="stable")       # [core, pos] -> local chunk
    T_sorted = np.sort(T_mat, axis=1)
    T_vec = T_sorted.max(axis=0)                          # [pos]
    NQ_vec = [_nq(int(t)) for t in T_vec]
    CO = np.concatenate([[0], np.cumsum([nq * WQ for nq in NQ_vec])])
    RO = np.concatenate([[0], np.cumsum(T_vec)])

    cols_arr = np.empty((N_CORES, P, int(CO[-1])), dtype=np.int16)
    rr_arr = np.empty((N_CORES, P, int(RO[-1])), dtype=np.float32)
    for core in range(N_CORES):
        for j in range(CHUNKS_PER_CORE):
            g = core * CHUNKS_PER_CORE + int(perm[core, j])
            T = int(T_vec[j])
            NQ = NQ_vec[j]
            cap = T * P
            s, e = bounds[g], bounds[g + 1]
            m = e - s
            cp = np.full(NQ * GQ, ZERO_ROW, dtype=np.int16)
            rp = np.full(cap, -1.0, dtype=np.float32)
            if m:
                cp[:m] = col_s[s:e]
                rp[:m] = (row_s[s:e] - g * P).astype(np.float32)
            # wrapped idx layout: piece q, element i -> [i % 16, i // 16],
            # replicated to all 128 partitions
            w = cp.reshape(NQ, WQ, 16).transpose(0, 2, 1)      # [NQ, 16, WQ]
            w = np.tile(w, (1, 8, 1))                          # [NQ, 128, WQ]
            cols_arr[core, :, int(CO[j]):int(CO[j + 1])] = (
                w.transpose(1, 0, 2).reshape(P, NQ * WQ))
            # rr for edge i -> [i % 128, i // 128]
            rr_arr[core, :, int(RO[j]):int(RO[j + 1])] = (
                rp.reshape(T, P).T)
    rr_arr = rr_arr.astype(NP_BF16)

    x_flat = np.zeros((NP_PAD, F), dtype=np.float32)
    x_flat[:n] = x.reshape(n, F)
    xg_full = np.ascontiguousarray(x_flat.astype(NP_BF16))

    xT = x_flat.T  # [512, 10240] for the root transform

    wsrelT = np.ascontiguousarray(np.asarray(W_scalar_rel, np.float32).T).astype(NP_BF16)
    wsrootT = np.ascontiguousarray(np.asarray(W_scalar_root, np.float32).T).astype(NP_BF16)
    wvrelT = np.ascontiguousarray(np.asarray(W_vector_rel, np.float32).T)
    wvrootT = np.ascontiguousarray(np.asarray(W_vector_root, np.float32).T)
    wvrel_packed = np.concatenate(
        [wvrelT[kc * P:(kc + 1) * P, :] for kc in range(3)], axis=1).astype(NP_BF16)
    wvroot_packed = np.concatenate(
        [wvrootT[kc * P:(kc + 1) * P, :] for kc in range(3)], axis=1).astype(NP_BF16)
    bias_t = np.ascontiguousarray(
        np.broadcast_to(np.asarray(b_scalar_root, np.float32), (P, H)))
    iota_t = np.ascontiguousarray(
        np.broadcast_to(np.arange(P, dtype=np.float32), (P, P))).astype(NP_BF16)
    ident_t = np.eye(P, dtype=np.float32).astype(NP_BF16)

    in_maps = []
    for core in range(N_CORES):
        base = core * NODES_PER_CORE
        # x^T columns permuted into this core's chunk processing order
        node_perm = (perm[core][:, None] * P + np.arange(P)[None, :]).reshape(-1)
        xTc = xT[:, base:base + NODES_PER_CORE][:, node_perm]  # [512, 1280]
        xTr = np.ascontiguousarray(
            xTc.reshape(4, P, NODES_PER_CORE).transpose(1, 0, 2)
               .reshape(P, 4 * NODES_PER_CORE)).astype(NP_BF16)
        in_maps.append({
            "xg": xg_full,
            "cols0": np.ascontiguousarray(cols_arr[core][:, :int(CO[1])]),
            "colsR": np.ascontiguousarray(cols_arr[core][:, int(CO[1]):]),
            "rr": np.ascontiguousarray(rr_arr[core]),
            "xt": xTr,
            "wsrel": wsrelT,
            "wsroot": wsrootT,
            "wvrel": wvrel_packed,
            "wvroot": wvroot_packed,
            "bias": bias_t,
            "iota": iota_t,
            "ident": ident_t,
        })

    nc = _get_program(T_vec)
    kw = {}
    if PROFILE["on"]:
        kw = dict(trace=True, trace_cores=PROFILE["trace_cores"])
    res = run_bass_kernel_spmd(nc, in_maps, list(range(N_CORES)), **kw)
    PROFILE["last"] = res

    out_full = np.empty((NP_PAD, F), dtype=np.float32)
    for core in range(N_CORES):
        o = np.asarray(res.results[core]["out"]).astype(np.float32)  # [1280, F]
        base = core * NODES_PER_CORE
        for j in range(CHUNKS_PER_CORE):
            g = int(perm[core, j])
            out_full[base + g * P: base + (g + 1) * P] = o[j * P:(j + 1) * P]
    return np.ascontiguousarray(
        out_full[:N_NODES].reshape(N_NODES, 4, H))
